# revision 9
# baseline (speedup 1.0000x reference)
"""Trainium2 Bass kernel for nn_Adapter (moe_routing).

Reference computation (per router m in [0,12), batch b in [0,32)):
    e = expert_index[m, b]
    z = x[b] @ down_w[m, e] + down_b[m, e]     # [S, D]
    z = z * sigmoid(z)                          # SiLU
    u[m, b] = z @ up_w[m, e]                    # [S, C]

Strategy:
  - Data-parallel over batch B=32 across 8 cores (4 batches per core).
  - Expert routing (the gather over expert_index) is done on HOST: each
    core receives the already-gathered per-(m,b) weight tables, laid out
    exactly as the SBUF tiles want them, pre-cast to bf16.
  - Device, per (b, m-pair): routers are processed two at a time packed
    into the 128x128 PE array:
      * down-proj: z^T[D=64,S] for m0 -> PE cols 0-63, m1 -> cols 64-127
        (col tiling), accumulating over 8 K-chunks of C=1024.
      * SiLU+bias on the combined [128,S] PSUM tile in one ScalarE op,
        output bf16.
      * up-proj: m0 -> PE rows 0-63, m1 -> rows 64-127 (row tiling),
        K=D=64, interleaved so consecutive instructions overlap in the
        array.
    PSUM evicted to SBUF as bf16 by ScalarE/VectorE; one fully
    contiguous 1MiB DMA out per (m,b) in [p, sc, c] layout which the
    host unpermutes.
  - Output returned to host as bf16, host upcasts to f32 and stitches.
"""

import os
import sys

sys.path.insert(0, "/opt/trn_rl_repo")

import numpy as np
import ml_dtypes

M, N_EXP, C, D = 12, 8, 1024, 64
B, S = 32, 512
NCORES = 8
BL = B // NCORES          # batches per core = 4
KC = C // 128             # contraction chunks for down-proj = 8
SC = S // 128             # output row chunks for up-proj = 4
JP = M // 2               # router pairs per batch = 6

BF16 = ml_dtypes.bfloat16

# set by test.py to collect the profile
TRACE = bool(os.environ.get("KERNEL_TRACE"))
last_results = None

_nc_cache = None


def _ensure_ntff_hook():
    """The agent image's `antenv` lacks `axon_hooks`, so the boot-time NTFF
    profile hook registration degrades silently and bass_utils' trace path
    crashes on import. Shim the module and install the ctypes hook."""
    import types

    if "antenv.axon_hooks" in sys.modules:
        return
    mod = types.ModuleType("antenv.axon_hooks")
    store = [None]
    mod.set_axon_ntff_profile_hook = lambda h: store.__setitem__(0, h)
    mod.get_axon_ntff_profile_hook = lambda: store[0]
    sys.modules["antenv.axon_hooks"] = mod
    try:
        import antenv

        antenv.axon_hooks = mod
    except ImportError:
        pass
    try:
        from trn_agent_boot.trn_boot import _ntff_profile_via_ctypes

        so_path = "/opt/axon/libaxon_pjrt.so"
        if os.path.exists(so_path):
            hook = _ntff_profile_via_ctypes(so_path)
            if hook is not None:
                mod.set_axon_ntff_profile_hook(hook)
    except Exception:
        pass


_ensure_ntff_hook()


def _build():
    import concourse.mybir as mybir
    from concourse import bacc, tile

    bf16 = mybir.dt.bfloat16
    f32 = mybir.dt.float32
    AF = mybir.ActivationFunctionType

    nc = bacc.Bacc(
        "TRN2", target_bir_lowering=False, debug=False, num_devices=NCORES
    )
    xt_d = nc.declare_dram_parameter("xt", [BL, 128, KC, S], bf16, isOutput=False)
    wd_d = nc.declare_dram_parameter("wd", [BL, 128, M, KC, D], bf16, isOutput=False)
    wu_d = nc.declare_dram_parameter("wu", [BL, 128, JP, C], bf16, isOutput=False)
    bias_d = nc.declare_dram_parameter("bias", [BL, 128, JP], f32, isOutput=False)
    # [m, b, p, sc, c]: fully contiguous per-(m,b) 1MiB DMA; host unpermutes
    out_d = nc.declare_dram_parameter("out", [M, BL, 128, SC, C], bf16, isOutput=True)

    with tile.TileContext(nc) as tc:
        with (
            tc.tile_pool(name="xin", bufs=2) as xin_pool,
            tc.tile_pool(name="wpool", bufs=2) as w_pool,
            tc.tile_pool(name="zt", bufs=3) as zt_pool,
            tc.tile_pool(name="usb", bufs=3) as u_pool,
            tc.tile_pool(name="pz", bufs=2, space="PSUM") as pz_pool,
            tc.tile_pool(name="pu", bufs=3, space="PSUM") as pu_pool,
        ):
            for b in range(BL):
                xt_sb = xin_pool.tile([128, KC, S], bf16, tag="xt")
                nc.gpsimd.dma_start(xt_sb[:], xt_d[b])
                wd_sb = w_pool.tile([128, M, KC, D], bf16, tag="wd")
                wu_sb = w_pool.tile([128, JP, C], bf16, tag="wu")
                bias_sb = w_pool.tile([128, JP], f32, tag="bias")
                nc.gpsimd.dma_start(bias_sb[:], bias_d[b])
                for j in range(JP):
                    nc.gpsimd.dma_start(
                        wd_sb[:, 2 * j : 2 * j + 2], wd_d[b, :, 2 * j : 2 * j + 2]
                    )
                    nc.gpsimd.dma_start(wu_sb[:, j], wu_d[b, :, j])

                for j in range(JP):
                    m0, m1 = 2 * j, 2 * j + 1
                    # down-proj, col-packed: m0 -> PE cols 0-63 -> psum
                    # partitions 0-63; m1 -> cols 64-127.
                    psum_z = pz_pool.tile([128, S], f32, tag="pz")
                    for k in range(KC):
                        nc.tensor.matmul(
                            psum_z[0:64, :],
                            lhsT=wd_sb[:, m0, k, :],
                            rhs=xt_sb[:, k, :],
                            start=(k == 0),
                            stop=(k == KC - 1),
                            tile_position=(0, 0),
                        )
                        nc.tensor.matmul(
                            psum_z[64:128, :],
                            lhsT=wd_sb[:, m1, k, :],
                            rhs=xt_sb[:, k, :],
                            start=(k == 0),
                            stop=(k == KC - 1),
                            tile_position=(0, 64),
                        )
                    # SiLU(z + bias) for both routers in one op, cast to bf16
                    zt_sb = zt_pool.tile([128, S], bf16, tag="zt")
                    nc.scalar.activation(
                        zt_sb[:], psum_z[:], AF.Silu, bias=bias_sb[:, j : j + 1]
                    )
                    # up-proj, row-packed: m0 -> PE rows 0-63, m1 -> rows
                    # 64-127, interleaved so the array works on both at once.
                    u0 = u_pool.tile([128, SC, C], bf16, tag="u0")
                    u1 = u_pool.tile([128, SC, C], bf16, tag="u1")
                    ev = j % 2
                    for sc in range(SC):
                        p0 = pu_pool.tile([128, C], f32, tag="pu")
                        p1 = pu_pool.tile([128, C], f32, tag="pu")
                        for cc in range(2):
                            nc.tensor.matmul(
                                p0[:, cc * 512 : (cc + 1) * 512],
                                lhsT=zt_sb[0:64, sc * 128 : (sc + 1) * 128],
                                rhs=wu_sb[0:64, j, cc * 512 : (cc + 1) * 512],
                                start=True,
                                stop=True,
                                tile_position=(0, 0),
                            )
                            nc.tensor.matmul(
                                p1[:, cc * 512 : (cc + 1) * 512],
                                lhsT=zt_sb[64:128, sc * 128 : (sc + 1) * 128],
                                rhs=wu_sb[64:128, j, cc * 512 : (cc + 1) * 512],
                                start=True,
                                stop=True,
                                tile_position=(64, 0),
                            )
                        for pt, ut in ((p0, u0), (p1, u1)):
                            dst = ut[:, sc, :]
                            if ev % 2 == 0:
                                nc.scalar.copy(dst, pt[:])
                            else:
                                nc.vector.tensor_copy(dst, pt[:])
                            ev += 1
                    for sc in range(SC):
                        nc.sync.dma_start(out_d[m0, b, :, sc], u0[:, sc])
                        nc.sync.dma_start(out_d[m1, b, :, sc], u1[:, sc])
    nc.compile()
    return nc


def _get_nc():
    global _nc_cache
    if _nc_cache is None:
        _nc_cache = _build()
    return _nc_cache


def kernel(x, expert_index, down_w, down_b, up_w):
    global last_results
    from concourse.bass_utils import run_bass_kernel_spmd

    x = np.asarray(x, dtype=np.float32)              # [B, S, C]
    idx = np.asarray(expert_index).astype(np.int64)  # [M, B]
    down_w = np.asarray(down_w, dtype=np.float32)    # [M, N, C, D]
    down_b = np.asarray(down_b, dtype=np.float32)    # [M, N, D]
    up_w = np.asarray(up_w, dtype=np.float32)        # [M, N, D, C]

    m_idx = np.arange(M)[:, None]
    wd_g = down_w[m_idx, idx]                        # [M, B, C, D]
    bb_g = down_b[m_idx, idx]                        # [M, B, D]
    wu_g = up_w[m_idx, idx]                          # [M, B, D, C]

    # xt[b, p, k, s] = x[b, s, k*128+p]
    xt = np.ascontiguousarray(
        x.transpose(0, 2, 1).reshape(B, KC, 128, S).transpose(0, 2, 1, 3)
    ).astype(BF16)
    # wd[b, p, m, k, d] = wd_g[m, b, k*128+p, d]
    wd = np.ascontiguousarray(
        wd_g.reshape(M, B, KC, 128, D).transpose(1, 3, 0, 2, 4)
    ).astype(BF16)
    # wu[b, p, j, c]: partitions 0-63 hold router 2j (d = p), partitions
    # 64-127 hold router 2j+1 (d = p-64)
    wu_p = wu_g.reshape(JP, 2, B, D, C).transpose(2, 1, 3, 0, 4)  # [B,2,D,JP,C]
    wu = np.ascontiguousarray(wu_p.reshape(B, 128, JP, C)).astype(BF16)
    # bias[b, p, j], same partition packing as wu
    bias_p = bb_g.reshape(JP, 2, B, D).transpose(2, 1, 3, 0)      # [B,2,D,JP]
    bias = np.ascontiguousarray(bias_p.reshape(B, 128, JP)).astype(np.float32)

    in_maps = []
    for core in range(NCORES):
        sl = slice(core * BL, (core + 1) * BL)
        in_maps.append(
            {
                "xt": xt[sl],
                "wd": wd[sl],
                "wu": wu[sl],
                "bias": bias[sl],
            }
        )

    nc = _get_nc()
    res = run_bass_kernel_spmd(
        nc, in_maps, core_ids=list(range(NCORES)), trace=TRACE
    )
    last_results = res

    out = np.empty((M, B, S, C), dtype=np.float32)
    for core in range(NCORES):
        sl = slice(core * BL, (core + 1) * BL)
        # dev out [M, BL, p, sc, c] -> [M, BL, s = sc*128+p, c]
        dev = res.results[core]["out"]
        out[:, sl] = dev.transpose(0, 1, 3, 2, 4).reshape(M, BL, S, C).astype(
            np.float32
        )
    return out


# revision 10
# speedup vs baseline: 1.1166x; 1.1166x over previous
"""Trainium2 Bass kernel for nn_Adapter (moe_routing).

Reference computation (per router m in [0,12), batch b in [0,32)):
    e = expert_index[m, b]
    z = x[b] @ down_w[m, e] + down_b[m, e]     # [S, D]
    z = z * sigmoid(z)                          # SiLU
    u[m, b] = z @ up_w[m, e]                    # [S, C]

Strategy:
  - Data-parallel over batch B=32 across 8 cores (4 batches per core).
  - Expert routing (the gather over expert_index) is done on HOST: each
    core receives the already-gathered per-(m,b) weight tables, laid out
    exactly as the SBUF tiles want them, pre-cast to bf16.
  - Device, per (b, m-pair): routers are processed two at a time packed
    into the 128x128 PE array:
      * down-proj: z^T[D=64,S] for m0 -> PE cols 0-63, m1 -> cols 64-127
        (col tiling), accumulating over 8 K-chunks of C=1024.
      * SiLU+bias on the combined [128,S] PSUM tile in one ScalarE op,
        output bf16.
      * up-proj: m0 -> PE rows 0-63, m1 -> rows 64-127 (row tiling),
        K=D=64, interleaved so consecutive instructions overlap in the
        array.
    PSUM evicted to SBUF as bf16 by ScalarE/VectorE; one fully
    contiguous 1MiB DMA out per (m,b) in [p, sc, c] layout which the
    host unpermutes.
  - Output returned to host as bf16, host upcasts to f32 and stitches.
"""

import os
import sys

sys.path.insert(0, "/opt/trn_rl_repo")

import numpy as np
import ml_dtypes

M, N_EXP, C, D = 12, 8, 1024, 64
B, S = 32, 512
NCORES = 8
BL = B // NCORES          # batches per core = 4
KC = C // 128             # contraction chunks for down-proj = 8
SC = S // 128             # output row chunks for up-proj = 4
JP = M // 2               # router pairs per batch = 6

BF16 = ml_dtypes.bfloat16

# set by test.py to collect the profile
TRACE = bool(os.environ.get("KERNEL_TRACE"))
last_results = None

_nc_cache = None


def _ensure_ntff_hook():
    """The agent image's `antenv` lacks `axon_hooks`, so the boot-time NTFF
    profile hook registration degrades silently and bass_utils' trace path
    crashes on import. Shim the module and install the ctypes hook."""
    import types

    if "antenv.axon_hooks" in sys.modules:
        return
    mod = types.ModuleType("antenv.axon_hooks")
    store = [None]
    mod.set_axon_ntff_profile_hook = lambda h: store.__setitem__(0, h)
    mod.get_axon_ntff_profile_hook = lambda: store[0]
    sys.modules["antenv.axon_hooks"] = mod
    try:
        import antenv

        antenv.axon_hooks = mod
    except ImportError:
        pass
    try:
        from trn_agent_boot.trn_boot import _ntff_profile_via_ctypes

        so_path = "/opt/axon/libaxon_pjrt.so"
        if os.path.exists(so_path):
            hook = _ntff_profile_via_ctypes(so_path)
            if hook is not None:
                mod.set_axon_ntff_profile_hook(hook)
    except Exception:
        pass


_ensure_ntff_hook()


def _build():
    import concourse.mybir as mybir
    from concourse import bacc, tile

    bf16 = mybir.dt.bfloat16
    f32 = mybir.dt.float32
    AF = mybir.ActivationFunctionType

    nc = bacc.Bacc(
        "TRN2", target_bir_lowering=False, debug=False, num_devices=NCORES
    )
    xt_d = nc.declare_dram_parameter("xt", [BL, 128, KC, S], bf16, isOutput=False)
    wd_d = nc.declare_dram_parameter("wd", [BL, 128, M, KC, D], bf16, isOutput=False)
    wu_d = nc.declare_dram_parameter("wu", [BL, 128, JP, C], bf16, isOutput=False)
    bias_d = nc.declare_dram_parameter("bias", [BL, 128, JP], f32, isOutput=False)
    # [m, b, p, sc, c]: fully contiguous per-(m,b) 1MiB DMA; host unpermutes
    out_d = nc.declare_dram_parameter("out", [M, BL, 128, SC, C], bf16, isOutput=True)

    with tile.TileContext(nc) as tc:
        with (
            tc.tile_pool(name="xin", bufs=2) as xin_pool,
            tc.tile_pool(name="wpool", bufs=2) as w_pool,
            tc.tile_pool(name="zt", bufs=3) as zt_pool,
            tc.tile_pool(name="usb", bufs=3) as u_pool,
            tc.tile_pool(name="pz", bufs=2, space="PSUM") as pz_pool,
            tc.tile_pool(name="pu", bufs=3, space="PSUM") as pu_pool,
        ):
            for b in range(BL):
                xt_sb = xin_pool.tile([128, KC, S], bf16, tag="xt")
                nc.gpsimd.dma_start(xt_sb[:], xt_d[b])
                wd_sb = w_pool.tile([128, M, KC, D], bf16, tag="wd")
                wu_sb = w_pool.tile([128, JP, C], bf16, tag="wu")
                bias_sb = w_pool.tile([128, JP], f32, tag="bias")
                if b == 0:
                    # first pair's weights land first so PE starts early
                    nc.gpsimd.dma_start(wd_sb[:, 0:2], wd_d[b, :, 0:2])
                    nc.gpsimd.dma_start(wu_sb[:, 0], wu_d[b, :, 0])
                    nc.gpsimd.dma_start(bias_sb[:], bias_d[b])
                    nc.gpsimd.dma_start(wd_sb[:, 2:M], wd_d[b, :, 2:M])
                    nc.gpsimd.dma_start(wu_sb[:, 1:JP], wu_d[b, :, 1:JP])
                else:
                    nc.gpsimd.dma_start(bias_sb[:], bias_d[b])
                    nc.gpsimd.dma_start(wd_sb[:], wd_d[b])
                    nc.gpsimd.dma_start(wu_sb[:], wu_d[b])

                for j in range(JP):
                    m0, m1 = 2 * j, 2 * j + 1
                    # down-proj, col-packed: m0 -> PE cols 0-63 -> psum
                    # partitions 0-63; m1 -> cols 64-127.
                    psum_z = pz_pool.tile([128, S], f32, tag="pz")
                    for k in range(KC):
                        nc.tensor.matmul(
                            psum_z[0:64, :],
                            lhsT=wd_sb[:, m0, k, :],
                            rhs=xt_sb[:, k, :],
                            start=(k == 0),
                            stop=(k == KC - 1),
                            tile_position=(0, 0),
                        )
                        nc.tensor.matmul(
                            psum_z[64:128, :],
                            lhsT=wd_sb[:, m1, k, :],
                            rhs=xt_sb[:, k, :],
                            start=(k == 0),
                            stop=(k == KC - 1),
                            tile_position=(0, 64),
                        )
                    # SiLU(z + bias) for both routers in one op, cast to bf16
                    zt_sb = zt_pool.tile([128, S], bf16, tag="zt")
                    nc.scalar.activation(
                        zt_sb[:], psum_z[:], AF.Silu, bias=bias_sb[:, j : j + 1]
                    )
                    # up-proj, row-packed: m0 -> PE rows 0-63, m1 -> rows
                    # 64-127, interleaved so the array works on both at once.
                    u0 = u_pool.tile([128, SC, C], bf16, tag="u0")
                    u1 = u_pool.tile([128, SC, C], bf16, tag="u1")
                    ev = j % 2
                    for sc in range(SC):
                        p0 = pu_pool.tile([128, C], f32, tag="pu")
                        p1 = pu_pool.tile([128, C], f32, tag="pu")
                        for cc in range(2):
                            nc.tensor.matmul(
                                p0[:, cc * 512 : (cc + 1) * 512],
                                lhsT=zt_sb[0:64, sc * 128 : (sc + 1) * 128],
                                rhs=wu_sb[0:64, j, cc * 512 : (cc + 1) * 512],
                                start=True,
                                stop=True,
                                tile_position=(0, 0),
                            )
                            nc.tensor.matmul(
                                p1[:, cc * 512 : (cc + 1) * 512],
                                lhsT=zt_sb[64:128, sc * 128 : (sc + 1) * 128],
                                rhs=wu_sb[64:128, j, cc * 512 : (cc + 1) * 512],
                                start=True,
                                stop=True,
                                tile_position=(64, 0),
                            )
                        for pt, ut in ((p0, u0), (p1, u1)):
                            dst = ut[:, sc, :]
                            if ev % 2 == 0:
                                nc.scalar.copy(dst, pt[:])
                            else:
                                nc.vector.tensor_copy(dst, pt[:])
                            ev += 1
                    for half in range(2):
                        hs = slice(half * 2, half * 2 + 2)
                        nc.sync.dma_start(out_d[m0, b, :, hs], u0[:, hs])
                        nc.sync.dma_start(out_d[m1, b, :, hs], u1[:, hs])
    nc.compile()
    return nc


def _get_nc():
    global _nc_cache
    if _nc_cache is None:
        _nc_cache = _build()
    return _nc_cache


def kernel(x, expert_index, down_w, down_b, up_w):
    global last_results
    from concourse.bass_utils import run_bass_kernel_spmd

    x = np.asarray(x, dtype=np.float32)              # [B, S, C]
    idx = np.asarray(expert_index).astype(np.int64)  # [M, B]
    down_w = np.asarray(down_w, dtype=np.float32)    # [M, N, C, D]
    down_b = np.asarray(down_b, dtype=np.float32)    # [M, N, D]
    up_w = np.asarray(up_w, dtype=np.float32)        # [M, N, D, C]

    m_idx = np.arange(M)[:, None]
    wd_g = down_w[m_idx, idx]                        # [M, B, C, D]
    bb_g = down_b[m_idx, idx]                        # [M, B, D]
    wu_g = up_w[m_idx, idx]                          # [M, B, D, C]

    # xt[b, p, k, s] = x[b, s, k*128+p]
    xt = np.ascontiguousarray(
        x.transpose(0, 2, 1).reshape(B, KC, 128, S).transpose(0, 2, 1, 3)
    ).astype(BF16)
    # wd[b, p, m, k, d] = wd_g[m, b, k*128+p, d]
    wd = np.ascontiguousarray(
        wd_g.reshape(M, B, KC, 128, D).transpose(1, 3, 0, 2, 4)
    ).astype(BF16)
    # wu[b, p, j, c]: partitions 0-63 hold router 2j (d = p), partitions
    # 64-127 hold router 2j+1 (d = p-64)
    wu_p = wu_g.reshape(JP, 2, B, D, C).transpose(2, 1, 3, 0, 4)  # [B,2,D,JP,C]
    wu = np.ascontiguousarray(wu_p.reshape(B, 128, JP, C)).astype(BF16)
    # bias[b, p, j], same partition packing as wu
    bias_p = bb_g.reshape(JP, 2, B, D).transpose(2, 1, 3, 0)      # [B,2,D,JP]
    bias = np.ascontiguousarray(bias_p.reshape(B, 128, JP)).astype(np.float32)

    in_maps = []
    for core in range(NCORES):
        sl = slice(core * BL, (core + 1) * BL)
        in_maps.append(
            {
                "xt": xt[sl],
                "wd": wd[sl],
                "wu": wu[sl],
                "bias": bias[sl],
            }
        )

    nc = _get_nc()
    res = run_bass_kernel_spmd(
        nc, in_maps, core_ids=list(range(NCORES)), trace=TRACE
    )
    last_results = res

    out = np.empty((M, B, S, C), dtype=np.float32)
    for core in range(NCORES):
        sl = slice(core * BL, (core + 1) * BL)
        # dev out [M, BL, p, sc, c] -> [M, BL, s = sc*128+p, c]
        dev = res.results[core]["out"]
        out[:, sl] = dev.transpose(0, 1, 3, 2, 4).reshape(M, BL, S, C).astype(
            np.float32
        )
    return out
